# revision 1
# baseline (speedup 1.0000x reference)
"""RNN-T Joiner kernel for Trainium2, data-parallel over (B, T) on 8 cores.

reference:
    logit = tanh(enc[:, :, None, :] + dec[:, None, :, :])   # (B,T,U,C)
    out   = einsum('btuc,vc->btuv', logit, W) + b           # (B,T,U,V)

Shapes (hardcoded): B=4, T=256, U=64, C=512, V=1024.

Sharding: core k handles b = k//2, t rows [ (k%2)*128, (k%2)*128+128 ).
W / bias replicated. No collectives.

Per-core device kernel (C on partitions for the logit):
  - logitT[c, t] = tanh(encT[c, t] + decT[c, u])  -- scalar engine, fused
    per-partition bias add.
  - out[t, v] accumulated over 4 c-chunks of K=128 matmuls; inputs bitcast
    to float32r (full PE rate at out-free-dim >= 256, fp32 data).
  - bias add fused into the PSUM->SBUF eviction on DVE.
  - out tile DMA'd straight to DRAM (2KB contiguous per partition).
"""

import numpy as np

B, T, U, C, V = 4, 256, 64, 512, 1024
NCORES = 8
TS = 128  # t rows per core
CCH = C // 128  # 4 contraction chunks
VH = V // 512  # 2 psum-width chunks

_CACHE = {}


def _build():
    from contextlib import ExitStack

    import concourse.bacc as bacc
    import concourse.mybir as mybir
    import concourse.tile as tile

    dt = mybir.dt
    f32 = dt.float32
    f32r = dt.float32r

    nc = bacc.Bacc("TRN2", target_bir_lowering=False, debug=False, num_devices=NCORES)
    enc_t = nc.declare_dram_parameter("enc_t", [C, TS], f32, isOutput=False)
    dec_t = nc.declare_dram_parameter("dec_t", [C, U], f32, isOutput=False)
    wt = nc.declare_dram_parameter("wt", [C, V], f32r, isOutput=False)
    bias_rep = nc.declare_dram_parameter("bias_rep", [128, V], f32, isOutput=False)
    out = nc.declare_dram_parameter("out", [TS, U, V], f32, isOutput=True)

    with tile.TileContext(nc) as tc, ExitStack() as ctx:
        const = ctx.enter_context(tc.tile_pool(name="const", bufs=1))
        logit_pool = ctx.enter_context(tc.tile_pool(name="logit", bufs=6))
        psum_pool = ctx.enter_context(tc.tile_pool(name="psum", bufs=4, space="PSUM"))
        out_pool = ctx.enter_context(tc.tile_pool(name="out", bufs=6))

        wt_sb = const.tile([128, CCH * V], f32r, tag="wt")
        enc_sb = const.tile([128, CCH * TS], f32, tag="enc")
        dec_sb = const.tile([128, CCH * U], f32, tag="dec")
        bias_sb = const.tile([128, V], f32, tag="bias")

        nc.sync.dma_start(
            enc_sb[:].rearrange("p (c t) -> p c t", c=CCH),
            enc_t[:].rearrange("(c p) t -> p c t", p=128),
        )
        nc.sync.dma_start(
            dec_sb[:].rearrange("p (c u) -> p c u", c=CCH),
            dec_t[:].rearrange("(c p) u -> p c u", p=128),
        )
        for c in range(CCH):
            nc.sync.dma_start(
                wt_sb[:, c * V : (c + 1) * V], wt[c * 128 : (c + 1) * 128, :]
            )
        nc.sync.dma_start(bias_sb[:], bias_rep[:])

        for u in range(U):
            lg = logit_pool.tile([128, CCH * TS], f32r, tag="lg")
            for c in range(CCH):
                nc.scalar.activation(
                    lg[:, c * TS : (c + 1) * TS],
                    enc_sb[:, c * TS : (c + 1) * TS],
                    mybir.ActivationFunctionType.Tanh,
                    bias=dec_sb[:, c * U + u : c * U + u + 1],
                )
            ps = psum_pool.tile([128, V], f32, tag="ps")
            for vh in range(VH):
                for c in range(CCH):
                    nc.tensor.matmul(
                        ps[:, vh * 512 : (vh + 1) * 512],
                        lhsT=lg[:, c * TS : (c + 1) * TS],
                        rhs=wt_sb[:, c * V + vh * 512 : c * V + vh * 512 + 512],
                        start=(c == 0),
                        stop=(c == CCH - 1),
                    )
            ob = out_pool.tile([128, V], f32, tag="ob")
            nc.vector.tensor_add(ob[:], ps[:], bias_sb[:])
            nc.sync.dma_start(out[:, u, :], ob[:])

    nc.finalize()
    return nc


def _get_nc():
    if "nc" not in _CACHE:
        _CACHE["nc"] = _build()
    return _CACHE["nc"]


def kernel(**inputs):
    enc = np.asarray(inputs["enc_out"], dtype=np.float32)
    dec = np.asarray(inputs["dec_out"], dtype=np.float32)
    W = np.asarray(inputs["W"], dtype=np.float32)
    b = np.asarray(inputs["b"], dtype=np.float32)

    nc = _get_nc()

    wt_np = np.ascontiguousarray(W.T)
    bias_np = np.ascontiguousarray(np.broadcast_to(b, (128, V)))
    in_maps = []
    for k in range(NCORES):
        bb, t0 = k // 2, (k % 2) * TS
        in_maps.append(
            {
                "enc_t": np.ascontiguousarray(enc[bb, t0 : t0 + TS, :].T),
                "dec_t": np.ascontiguousarray(dec[bb].T),
                "wt": wt_np,
                "bias_rep": bias_np,
            }
        )

    from concourse.bass_utils import run_bass_kernel_spmd

    res = run_bass_kernel_spmd(nc, in_maps, list(range(NCORES)))
    _CACHE["last_result"] = res

    out = np.empty((B, T, U, V), np.float32)
    for k in range(NCORES):
        bb, t0 = k // 2, (k % 2) * TS
        out[bb, t0 : t0 + TS] = res.results[k]["out"]
    return out



# revision 2
# speedup vs baseline: 1.0508x; 1.0508x over previous
"""RNN-T Joiner kernel for Trainium2, data-parallel over (B, T) on 8 cores.

reference:
    logit = tanh(enc[:, :, None, :] + dec[:, None, :, :])   # (B,T,U,C)
    out   = einsum('btuc,vc->btuv', logit, W) + b           # (B,T,U,V)

Shapes (hardcoded): B=4, T=256, U=64, C=512, V=1024.

Sharding: core k handles b = k//2, t rows [ (k%2)*128, (k%2)*128+128 ).
W / bias replicated. No collectives.

Per-core device kernel (C on partitions for the logit):
  - logitT[c, t] = tanh(encT[c, t] + decT[c, u])  -- scalar engine, fused
    per-partition bias add, fp16 output.
  - out[t, v] accumulated over 4 c-chunks of K=128 fp16 matmuls (full PE
    rate, half the SBUF/DMA bytes of fp32).
  - PE pre-warmed with dummy matmuls during the input-DMA head so the
    DVFS ramp is spent before real work arrives.
  - bias add fused into the PSUM->SBUF eviction on DVE, fp16 output.
  - output written fp16 (host upcasts), 4 u-steps batched per DMA so
    descriptors are 8KB contiguous.
"""

import numpy as np

B, T, U, C, V = 4, 256, 64, 512, 1024
NCORES = 8
TS = 128  # t rows per core
CCH = C // 128  # 4 contraction chunks
VH = V // 512  # 2 psum-width chunks
UB = 4  # u-steps batched per output DMA
NWARM_BIG = 8  # 512-row warmup matmuls (DVFS ramp)
NWARM_SMALL = 16  # 128-row warmup matmuls (fine-grained drain)

_CACHE = {}


def _build():
    from contextlib import ExitStack

    import concourse.bacc as bacc
    import concourse.mybir as mybir
    import concourse.tile as tile

    dt = mybir.dt
    f32 = dt.float32
    f16 = dt.float16

    nc = bacc.Bacc("TRN2", target_bir_lowering=False, debug=False, num_devices=NCORES)
    # all inputs pre-laid-out on host so every DMA is >=1KB contiguous
    # per partition: X_l[p, c*N+n] = X[c*128+p, n]
    enc_l = nc.declare_dram_parameter("enc_l", [128, CCH * TS], f32, isOutput=False)
    dec_l = nc.declare_dram_parameter("dec_l", [128, CCH * U], f32, isOutput=False)
    wt = nc.declare_dram_parameter("wt", [128, CCH * V], f16, isOutput=False)
    bias_rep = nc.declare_dram_parameter("bias_rep", [128, V], f32, isOutput=False)
    out = nc.declare_dram_parameter("out", [TS, U, V], f16, isOutput=True)

    with tile.TileContext(nc) as tc, ExitStack() as ctx:
        const = ctx.enter_context(tc.tile_pool(name="const", bufs=1))
        logit_pool = ctx.enter_context(tc.tile_pool(name="logit", bufs=6))
        psum_pool = ctx.enter_context(tc.tile_pool(name="psum", bufs=3, space="PSUM"))
        warm_pool = ctx.enter_context(tc.tile_pool(name="warm", bufs=1, space="PSUM"))
        out_pool = ctx.enter_context(tc.tile_pool(name="out", bufs=3))

        warm_sb = const.tile([128, 512], f16, tag="warm")
        wt_sb = const.tile([128, CCH * V], f16, tag="wt")
        enc_sb = const.tile([128, CCH * TS], f32, tag="enc")
        dec_sb = const.tile([128, CCH * U], f32, tag="dec")
        bias_sb = const.tile([128, V], f32, tag="bias")

        # PE warmup: no data deps, runs while input DMAs stream in.
        nc.vector.memset(warm_sb[:], 0.0)
        warm_ps = warm_pool.tile([128, 512], f32, tag="wps")
        for _ in range(NWARM_BIG):
            nc.tensor.matmul(
                warm_ps[:], lhsT=warm_sb[:, :128], rhs=warm_sb[:], start=True, stop=True
            )
        for _ in range(NWARM_SMALL):
            nc.tensor.matmul(
                warm_ps[:, :128],
                lhsT=warm_sb[:, :128],
                rhs=warm_sb[:, :128],
                start=True,
                stop=True,
            )

        nc.sync.dma_start(enc_sb[:], enc_l[:])
        nc.sync.dma_start(dec_sb[:], dec_l[:])
        for c in range(CCH):
            nc.sync.dma_start(wt_sb[:, c * V : (c + 1) * V], wt[:, c * V : (c + 1) * V])
        nc.sync.dma_start(bias_sb[:], bias_rep[:])

        for ub in range(U // UB):
            ob = out_pool.tile([128, UB * V], f16, tag="ob")
            for j in range(UB):
                u = ub * UB + j
                lg = logit_pool.tile([128, CCH * TS], f16, tag="lg")
                for c in range(CCH):
                    nc.scalar.activation(
                        lg[:, c * TS : (c + 1) * TS],
                        enc_sb[:, c * TS : (c + 1) * TS],
                        mybir.ActivationFunctionType.Tanh,
                        bias=dec_sb[:, c * U + u : c * U + u + 1],
                    )
                ps = psum_pool.tile([128, V], f32, tag="ps")
                for c in range(CCH):
                    for vh in range(VH):
                        nc.tensor.matmul(
                            ps[:, vh * 512 : (vh + 1) * 512],
                            lhsT=lg[:, c * TS : (c + 1) * TS],
                            rhs=wt_sb[:, c * V + vh * 512 : c * V + vh * 512 + 512],
                            start=(c == 0),
                            stop=(c == CCH - 1),
                        )
                nc.vector.tensor_add(ob[:, j * V : (j + 1) * V], ps[:], bias_sb[:])
            nc.sync.dma_start(out[:, ub * UB : (ub + 1) * UB, :], ob[:])

    nc.finalize()
    return nc


def _get_nc():
    if "nc" not in _CACHE:
        _CACHE["nc"] = _build()
    return _CACHE["nc"]


def _chunked(x):
    # [C, N] -> [128, CCH*N] with row p, col c*N+n = x[c*128+p, n]
    n = x.shape[1]
    return np.ascontiguousarray(
        x.reshape(CCH, 128, n).transpose(1, 0, 2).reshape(128, CCH * n)
    )


def kernel(**inputs):
    enc = np.asarray(inputs["enc_out"], dtype=np.float32)
    dec = np.asarray(inputs["dec_out"], dtype=np.float32)
    W = np.asarray(inputs["W"], dtype=np.float32)
    b = np.asarray(inputs["b"], dtype=np.float32)

    nc = _get_nc()

    wt_np = _chunked(W.T.astype(np.float32)).astype(np.float16)
    bias_np = np.ascontiguousarray(np.broadcast_to(b, (128, V)), dtype=np.float32)
    in_maps = []
    for k in range(NCORES):
        bb, t0 = k // 2, (k % 2) * TS
        in_maps.append(
            {
                "enc_l": _chunked(np.ascontiguousarray(enc[bb, t0 : t0 + TS, :].T)),
                "dec_l": _chunked(np.ascontiguousarray(dec[bb].T)),
                "wt": wt_np,
                "bias_rep": bias_np,
            }
        )

    from concourse.bass_utils import run_bass_kernel_spmd

    res = run_bass_kernel_spmd(nc, in_maps, list(range(NCORES)))
    _CACHE["last_result"] = res

    out = np.empty((B, T, U, V), np.float32)
    for k in range(NCORES):
        bb, t0 = k // 2, (k % 2) * TS
        out[bb, t0 : t0 + TS] = res.results[k]["out"].astype(np.float32)
    return out


# revision 8
# speedup vs baseline: 1.0827x; 1.0304x over previous
"""RNN-T Joiner kernel for Trainium2, data-parallel over (B, T) on 8 cores.

reference:
    logit = tanh(enc[:, :, None, :] + dec[:, None, :, :])   # (B,T,U,C)
    out   = einsum('btuc,vc->btuv', logit, W) + b           # (B,T,U,V)

Shapes (hardcoded): B=4, T=256, U=64, C=512, V=1024.

Sharding: core k handles b = k//2, t rows [ (k%2)*128, (k%2)*128+128 ).
W / bias replicated. No collectives.

Per-core device kernel (C on partitions for the logit):
  - logitT[c, t] = tanh(encT[c, t] + decT[c, u])  -- scalar engine, fused
    per-partition bias add, fp16 output.
  - out[t, v] accumulated over 4 c-chunks of K=128 fp16 matmuls (full PE
    rate, half the SBUF/DMA bytes of fp32).
  - PE pre-warmed with dummy matmuls during the input-DMA head so the
    DVFS ramp is spent before real work arrives.
  - bias add fused into the PSUM->SBUF eviction on DVE, fp16 output.
  - output written fp16 (host upcasts), 4 u-steps batched per DMA so
    descriptors are 8KB contiguous.
"""

import numpy as np

B, T, U, C, V = 4, 256, 64, 512, 1024
NCORES = 8
TS = 128  # t rows per core
CCH = C // 128  # 4 contraction chunks
VH = V // 512  # 2 psum-width chunks
UB = 2  # u-steps batched per output DMA
NWARM_BIG = 8  # 512-row warmup matmuls (DVFS ramp)
NWARM_SMALL = 10  # 128-row warmup matmuls (fine-grained drain)

_CACHE = {}


def _build():
    from contextlib import ExitStack

    import concourse.bacc as bacc
    import concourse.mybir as mybir
    import concourse.tile as tile

    dt = mybir.dt
    f32 = dt.float32
    f16 = dt.float16

    nc = bacc.Bacc("TRN2", target_bir_lowering=False, debug=False, num_devices=NCORES)
    # all inputs pre-laid-out on host so every DMA is >=1KB contiguous
    # per partition: X_l[p, c*N+n] = X[c*128+p, n]
    enc_l = nc.declare_dram_parameter("enc_l", [128, CCH * TS], f16, isOutput=False)
    dec_l = nc.declare_dram_parameter("dec_l", [128, CCH * U], f16, isOutput=False)
    wt = nc.declare_dram_parameter("wt", [128, CCH * V], f16, isOutput=False)
    bias_rep = nc.declare_dram_parameter("bias_rep", [128, V], f32, isOutput=False)
    out = nc.declare_dram_parameter("out", [TS, U, V], f16, isOutput=True)

    with tile.TileContext(nc) as tc, ExitStack() as ctx:
        const = ctx.enter_context(tc.tile_pool(name="const", bufs=1))
        logit_pool = ctx.enter_context(tc.tile_pool(name="logit", bufs=6))
        psum_pool = ctx.enter_context(tc.tile_pool(name="psum", bufs=3, space="PSUM"))
        warm_pool = ctx.enter_context(tc.tile_pool(name="warm", bufs=1, space="PSUM"))
        out_pool = ctx.enter_context(tc.tile_pool(name="out", bufs=3))

        warm_sb = const.tile([128, 512], f16, tag="warm")
        wt_sb = const.tile([128, CCH * V], f16, tag="wt")
        enc_sb = const.tile([128, CCH * TS], f16, tag="enc")
        dec_sb = const.tile([128, CCH * U], f16, tag="dec")
        bias_sb = const.tile([128, V], f32, tag="bias")

        # PE warmup: no data deps, runs while input DMAs stream in.
        nc.vector.memset(warm_sb[:], 0.0)
        warm_ps = warm_pool.tile([128, 512], f32, tag="wps")
        for _ in range(NWARM_BIG):
            nc.tensor.matmul(
                warm_ps[:], lhsT=warm_sb[:, :128], rhs=warm_sb[:], start=True, stop=True
            )
        for _ in range(NWARM_SMALL):
            nc.tensor.matmul(
                warm_ps[:, :128],
                lhsT=warm_sb[:, :128],
                rhs=warm_sb[:, :128],
                start=True,
                stop=True,
            )

        nc.sync.dma_start(enc_sb[:], enc_l[:])
        nc.sync.dma_start(dec_sb[:], dec_l[:])
        # wt + bias on the Activation-engine HWDGE queues so they stream
        # in parallel with enc/dec on the SP queues.
        for c in range(CCH):
            nc.scalar.dma_start(
                wt_sb[:, c * V : (c + 1) * V], wt[:, c * V : (c + 1) * V]
            )
        nc.scalar.dma_start(bias_sb[:], bias_rep[:])

        for ub in range(U // UB):
            ob = out_pool.tile([128, UB * V], f16, tag="ob")
            for j in range(UB):
                u = ub * UB + j
                # one tile per c-chunk so the c=0 matmul only waits on the
                # first activation, not all four
                lgs = []
                for c in range(CCH):
                    lg_c = logit_pool.tile([128, TS], f16, tag=f"lg{c}", name=f"lg{c}")
                    lgs.append(lg_c)
                for c in range(CCH):
                    nc.scalar.activation(
                        lgs[c][:],
                        enc_sb[:, c * TS : (c + 1) * TS],
                        mybir.ActivationFunctionType.Tanh,
                        bias=dec_sb[:, c * U + u : c * U + u + 1],
                    )
                ps = psum_pool.tile([128, V], f32, tag="ps")
                for c in range(CCH):
                    for vh in range(VH):
                        nc.tensor.matmul(
                            ps[:, vh * 512 : (vh + 1) * 512],
                            lhsT=lgs[c][:],
                            rhs=wt_sb[:, c * V + vh * 512 : c * V + vh * 512 + 512],
                            start=(c == 0),
                            stop=(c == CCH - 1),
                        )
                nc.vector.tensor_add(ob[:, j * V : (j + 1) * V], ps[:], bias_sb[:])
            nc.sync.dma_start(out[:, ub * UB : (ub + 1) * UB, :], ob[:])

    nc.finalize()
    return nc


def _get_nc():
    if "nc" not in _CACHE:
        _CACHE["nc"] = _build()
    return _CACHE["nc"]


def _chunked(x):
    # [C, N] -> [128, CCH*N] with row p, col c*N+n = x[c*128+p, n]
    n = x.shape[1]
    return np.ascontiguousarray(
        x.reshape(CCH, 128, n).transpose(1, 0, 2).reshape(128, CCH * n)
    )


def kernel(**inputs):
    enc = np.asarray(inputs["enc_out"], dtype=np.float32)
    dec = np.asarray(inputs["dec_out"], dtype=np.float32)
    W = np.asarray(inputs["W"], dtype=np.float32)
    b = np.asarray(inputs["b"], dtype=np.float32)

    nc = _get_nc()

    wt_np = _chunked(W.T.astype(np.float32)).astype(np.float16)
    bias_np = np.ascontiguousarray(np.broadcast_to(b, (128, V)), dtype=np.float32)
    in_maps = []
    for k in range(NCORES):
        bb, t0 = k // 2, (k % 2) * TS
        in_maps.append(
            {
                "enc_l": _chunked(np.ascontiguousarray(enc[bb, t0 : t0 + TS, :].T)).astype(np.float16),
                "dec_l": _chunked(np.ascontiguousarray(dec[bb].T)).astype(np.float16),
                "wt": wt_np,
                "bias_rep": bias_np,
            }
        )

    from concourse.bass_utils import run_bass_kernel_spmd

    res = run_bass_kernel_spmd(nc, in_maps, list(range(NCORES)))
    _CACHE["last_result"] = res

    out = np.empty((B, T, U, V), np.float32)
    for k in range(NCORES):
        bb, t0 = k // 2, (k % 2) * TS
        out[bb, t0 : t0 + TS] = res.results[k]["out"].astype(np.float32)
    return out
